# revision 18
# baseline (speedup 1.0000x reference)
"""BiSS2D (binary VMamba block) Trainium2 kernel, 8-core SPMD.

Sharding: core c = (batch b = c//2, half hf = c%2).
 - Each core-pair shares one batch; per-core d-half (96 of 192 channels)
   for conv output + all 4 direction scans; out-proj split by dm-half.
 - Two 2-core AllGathers: sign(xconv) exchange, y exchange.
All sign-quantized matmuls run as exact {0,1}x{+-1} bf16 matmuls with
f32 scale/bias applied at PSUM evacuation.
"""
import sys

sys.path.insert(0, "/opt/trn_rl_repo")
import numpy as np
import ml_dtypes

import concourse.mybir as mybir
import concourse.tile as tile
from concourse import bacc
from concourse.ap import AP
from concourse.bass_utils import run_bass_kernel_spmd

f32 = mybir.dt.float32
bf16 = mybir.dt.bfloat16
AF = mybir.ActivationFunctionType
OP = mybir.AluOpType

B, H, W, DM = 4, 64, 64, 96
DN, N, R, K = 192, 16, 6, 4
L = H * W            # 4096
DH = DN // 2         # 96 per-core channels
NT = DH // 8         # 12 scan tiles, partitions = (doff 8 x n 16)
LC = 512
NLC = L // LC        # 8
EPS = 1e-5

_cache = {}


class Cols:
    in_scale = [0, 1, 2, 3]
    in_bias = [4, 5, 6, 7]
    neg_move0 = [8, 9]
    conv_scale = 10
    conv_bias = 11
    prelu = 12
    rp_b1 = 13
    dts_scale = [14, 15, 16, 17]
    dts_bias = [18, 19, 20, 21]
    bc_scale = [22, 23, 24, 25]
    bc_bias = [26, 27, 28, 29]
    d6_scale = [30, 31, 32, 33]
    d6_bias = [34, 35, 36, 37]
    dsum = 38
    lnw = [39, 40]
    lnb = [41, 42]
    out_scale = 43
    out_bias = 44
    NCOL = 45


def _revfree(ap, n):
    return AP(ap.tensor, ap.offset + (n - 1), [ap.ap[0], [-1, n]])


def _geo(ap, k):
    """Direction-k free-dim re-order of a [P, 4096] AP (canonical -> scan)."""
    if k == 0:
        return ap
    if k == 1:
        return AP(ap.tensor, ap.offset, [ap.ap[0], [1, 64], [64, 64]])
    if k == 2:
        return _revfree(ap, L)
    return AP(ap.tensor, ap.offset + (L - 1), [ap.ap[0], [-1, 64], [-64, 64]])


def _permpair(dst_ap, src_ap):
    """Engine-op AP pair implementing the (h,w)->(w,h) free re-order."""
    do = AP(dst_ap.tensor, dst_ap.offset, [dst_ap.ap[0], [64, 64], [1, 64]])
    si = AP(src_ap.tensor, src_ap.offset, [src_ap.ap[0], [1, 64], [64, 64]])
    return do, si


def _half(ap, a, n=2048):
    """Column-slice [a, a+n) of a [P, L]-style AP (handles reversed APs)."""
    st = ap.ap[-1][0]
    if st == 1:
        return AP(ap.tensor, ap.offset + a, ap.ap[:-1] + [[1, n]])
    assert st == -1
    return AP(ap.tensor, ap.offset - a, ap.ap[:-1] + [[-1, n]])


def _dma2(nc, dst, src, n=L):
    for a in range(0, n, 2048):
        nc.sync.dma_start(_half(dst, a), _half(src, a))


def _build_nc():
    nc = bacc.Bacc(num_devices=8)
    I, O, INT = "ExternalInput", "ExternalOutput", "Internal"
    x_t = nc.dram_tensor("x_t", [96, L], f32, kind=I)
    w_in_T = nc.dram_tensor("w_in_T", [96, 384], bf16, kind=I)
    conv_lhsT = nc.dram_tensor("conv_lhsT", [9, DN, DH], bf16, kind=I)
    wl_T = nc.dram_tensor("wl_T", [K, DN, 38], bf16, kind=I)
    wd_T = nc.dram_tensor("wd_T", [K, R, DH], bf16, kind=I)
    wout_T = nc.dram_tensor("wout_T", [DN, 48], bf16, kind=I)
    params = nc.dram_tensor("params", [128, Cols.NCOL], f32, kind=I)
    a_pat = nc.dram_tensor("a_pat", [128, K * NT], f32, kind=I)
    bigpat = nc.dram_tensor("bigpat", [128, 184], bf16, kind=I)

    xs_hbm = nc.dram_tensor("xs_hbm", [K, DN, L], bf16, kind=INT)
    dts_hbm = nc.dram_tensor("dts_hbm", [K, R, L], bf16, kind=INT)
    yk_hbm = nc.dram_tensor("yk_hbm", [K, DH, L], f32, kind=INT)
    cc1_in = nc.dram_tensor("cc1_in", [DH, L], bf16, kind=INT)
    cc1_out = nc.dram_tensor("cc1_out", [DN, L], bf16, kind=INT)
    cc2_in = nc.dram_tensor("cc2_in", [DH, L], f32, kind=INT)
    cc2_out = nc.dram_tensor("cc2_out", [DN, L], f32, kind=INT)
    out_h = nc.dram_tensor("out_h", [48, L], f32, kind=O)

    groups = [[0, 1], [2, 3], [4, 5], [6, 7]]

    with tile.TileContext(nc) as tc:
        with (
            tc.tile_pool(name="const", bufs=1) as cp,
            tc.tile_pool(name="keep", bufs=1) as kp,
            tc.tile_pool(name="mm", bufs=2, space="PSUM") as mp,
            tc.tile_pool(name="ypsum", bufs=1, space="PSUM") as yp,
        ):
            # ---- constants
            prm = cp.tile([128, Cols.NCOL], f32)
            nc.sync.dma_start(prm[:], params[:])
            asb = cp.tile([128, K * NT], f32)
            nc.sync.dma_start(asb[:], a_pat[:])
            big = cp.tile([128, 184], bf16)
            nc.sync.dma_start(big[:], bigpat[:])
            winsb = cp.tile([96, 384], bf16)
            nc.sync.dma_start(winsb[:], w_in_T[:])
            cw0 = cp.tile([128, 9 * DH], bf16)
            nc.sync.dma_start(cw0[:], AP(conv_lhsT, 0, [[DH, 128], [DN * DH, 9], [1, DH]]))
            cw1 = cp.tile([64, 9 * DH], bf16)
            nc.sync.dma_start(cw1[:], AP(conv_lhsT, 128 * DH, [[DH, 64], [DN * DH, 9], [1, DH]]))
            wl0 = cp.tile([128, K * 38], bf16)
            nc.sync.dma_start(wl0[:], AP(wl_T, 0, [[38, 128], [DN * 38, K], [1, 38]]))
            wl1 = cp.tile([64, K * 38], bf16)
            nc.sync.dma_start(wl1[:], AP(wl_T, 128 * 38, [[38, 64], [DN * 38, K], [1, 38]]))
            wdsb = cp.tile([R, K * DH], bf16)
            nc.sync.dma_start(wdsb[:], AP(wd_T, 0, [[DH, R], [R * DH, K], [1, DH]]))
            wo0 = cp.tile([128, 48], bf16)
            nc.sync.dma_start(wo0[:], wout_T[0:128, :])
            wo1 = cp.tile([64, 48], bf16)
            nc.sync.dma_start(wo1[:], wout_T[128:DN, :])
            onesb = cp.tile([128, 1], bf16)
            nc.vector.memset(onesb[:], 1.0)
            onebc = cp.tile([1, 128], bf16)
            nc.vector.memset(onebc[:], 1.0)

            pc = lambda c, n=128: prm[0:n, c:c + 1]

            # persistent across phases
            z0 = kp.tile([128, L], bf16)
            z1 = kp.tile([64, L], bf16)
            xconv = kp.tile([96, L], f32)

            # ---- phase A+B: in-proj + conv --------------------------------
            with tc.tile_pool(name="ab", bufs=1) as ab, \
                 tc.tile_pool(name="ab2", bufs=2) as ab2:
                xt = ab.tile([96, L], f32)
                nc.sync.dma_start(xt[:], x_t[:])
                sx = ab.tile([96, L], bf16)
                nc.vector.tensor_scalar(sx[:], xt[:], 0.0, None, op0=OP.is_gt)
                xp0 = ab.tile([128, L], f32)
                xp1 = ab.tile([64, L], f32)
                mchunks = [(0, 128, xp0, 0, False), (128, 64, xp1, 1, False),
                           (192, 128, z0, 2, True), (320, 64, z1, 3, True)]
                for m0, msz, dst, mi, is_z in mchunks:
                    for fc in range(NLC):
                        fs = slice(fc * LC, (fc + 1) * LC)
                        ps = mp.tile([msz, LC], f32, tag="mmA")
                        nc.tensor.matmul(ps[:], winsb[:, m0:m0 + msz], sx[:, fs],
                                         start=True, stop=True)
                        if is_z:
                            zl = ab2.tile([msz, LC], f32, tag="zl")
                            nc.scalar.activation(zl[:], ps[:], AF.Identity,
                                                 bias=pc(Cols.in_bias[mi], msz),
                                                 scale=pc(Cols.in_scale[mi], msz))
                            nc.vector.tensor_scalar(dst[:, fs], zl[:], 0.0, None,
                                                    op0=OP.is_gt)
                        else:
                            nc.scalar.activation(dst[:, fs], ps[:], AF.Identity,
                                                 bias=pc(Cols.in_bias[mi], msz),
                                                 scale=pc(Cols.in_scale[mi], msz))

                LP = L + 192
                tmid0 = ab.tile([128, LP], bf16)
                tmid1 = ab.tile([64, LP], bf16)
                tm10 = ab.tile([128, LP], bf16)
                tm11 = ab.tile([64, LP], bf16)
                tp10 = ab.tile([128, LP], bf16)
                tp11 = ab.tile([64, LP], bf16)
                for t_, xsrc, n_, c_ in ((tmid0, xp0, 128, 0), (tmid1, xp1, 64, 1)):
                    nc.vector.memset(t_[:], 0.5)
                    nc.vector.tensor_scalar(t_[:, 96:96 + L], xsrc[:],
                                            pc(Cols.neg_move0[c_], n_), None, op0=OP.is_gt)
                for dst, src in ((tm10, tmid0), (tm11, tmid1), (tp10, tmid0), (tp11, tmid1)):
                    nc.vector.tensor_copy(dst[:], src[:])
                for t_ in (tm10, tm11):
                    nc.vector.memset(AP(t_[:].tensor, t_[:].offset + 96 + 63,
                                        [t_[:].ap[0], [64, 64]]), 0.5)
                for t_ in (tp10, tp11):
                    nc.vector.memset(AP(t_[:].tensor, t_[:].offset + 96,
                                        [t_[:].ap[0], [64, 64]]), 0.5)

                c01m = ab.tile([96, L], bf16)
                for fc in range(NLC):
                    fs = slice(fc * LC, (fc + 1) * LC)
                    ps = mp.tile([96, LC], f32, tag="mmA")
                    idx = 0
                    for ky in range(3):
                        for kx in range(3):
                            shift = (ky - 1) * 64 + (kx - 1)
                            s0, s1 = ((tm10, tm11), (tmid0, tmid1), (tp10, tp11))[kx]
                            o = 96 + fc * LC + shift
                            nc.tensor.matmul(ps[:], cw0[:, idx * DH:(idx + 1) * DH],
                                             s0[:, o:o + LC], start=(idx == 0), stop=False)
                            nc.tensor.matmul(ps[:], cw1[:, idx * DH:(idx + 1) * DH],
                                             s1[:, o:o + LC], start=False, stop=(idx == 8))
                            idx += 1
                    co = ab2.tile([96, LC], f32, tag="convo")
                    nc.scalar.activation(co[:], ps[:], AF.Identity,
                                         bias=pc(Cols.conv_bias, 96),
                                         scale=pc(Cols.conv_scale, 96))
                    pos = ab2.tile([96, LC], f32, tag="cpos")
                    nc.vector.tensor_scalar(pos[:], co[:], 0.0, None, op0=OP.max)
                    neg = ab2.tile([96, LC], f32, tag="cneg")
                    nc.vector.tensor_scalar(neg[:], co[:], 0.0, pc(Cols.prelu, 96),
                                            op0=OP.min, op1=OP.mult)
                    nc.vector.tensor_add(pos[:], pos[:], neg[:])
                    nc.vector.scalar_tensor_tensor(pos[:], pos[:], pc(Cols.rp_b1, 96),
                                                   xp0[0:96, fs], op0=OP.add, op1=OP.add)
                    es = ab2.tile([96, LC], f32, tag="es")
                    nc.scalar.activation(es[:], pos[:], AF.Exp, scale=-1.0)
                    nc.vector.tensor_scalar(es[:], es[:], 1.0, None, op0=OP.add)
                    nc.vector.reciprocal_approx_fast(es[:], es[:])
                    nc.vector.tensor_mul(xconv[:, fs], pos[:], es[:])
                    nc.vector.tensor_scalar(c01m[:, fs], xconv[:, fs], 0.0, None, op0=OP.is_gt)

                _dma2(nc, cc1_in[:], c01m[:])

            nc.gpsimd.collective_compute("AllGather", OP.bypass, replica_groups=groups,
                                         ins=[cc1_in[:]], outs=[cc1_out[:]])

            # ---- phase C-E: per-direction projections + scan --------------
            with tc.tile_pool(name="cde", bufs=1) as cde, \
                 tc.tile_pool(name="cde2", bufs=2) as cde2:
                xcb = cde.tile([96, L], bf16)
                nc.vector.tensor_copy(xcb[:], xconv[:])
                with tc.tile_pool(name="xsb", bufs=1) as xsb:
                    xcf0 = xsb.tile([128, L], bf16)
                    xcf1 = xsb.tile([64, L], bf16)
                    _dma2(nc, xcf0[:], cc1_out[0:128, :])
                    _dma2(nc, xcf1[:], cc1_out[128:DN, :])
                    xt0 = xsb.tile([128, L], bf16)
                    xt1 = xsb.tile([64, L], bf16)
                    for dst_t, src_t in ((xt0, xcf0), (xt1, xcf1)):
                        do_, si_ = _permpair(dst_t[:], src_t[:])
                        nc.vector.tensor_copy(do_, si_)
                    for k, (s0, s1) in enumerate(((xcf0, xcf1), (xt0, xt1),
                                                 (xcf0, xcf1), (xt0, xt1))):
                        rev = k >= 2
                        for s_t, off, nrow in ((s0, k * DN * L, 128),
                                               (s1, k * DN * L + 128 * L, 64)):
                            dst = AP(xs_hbm, off, [[L, nrow], [1, L]])
                            sap = _revfree(s_t[:], L) if rev else s_t[:]
                            _dma2(nc, dst, sap)

                sc3cm = tc.tile_pool(name="sc3", bufs=3)
                sc3 = sc3cm.__enter__()
                uk1 = cde.tile([96, L], bf16)
                do_, si_ = _permpair(uk1[:], xcb[:])
                nc.vector.tensor_copy(do_, si_)
                uk2 = cde.tile([96, L], bf16)
                _dma2(nc, uk2[:], _revfree(xcb[:], L))
                uk3 = cde.tile([96, L], bf16)
                _dma2(nc, uk3[:], _revfree(uk1[:], L))
                uks = (xcb, uk1, uk2, uk3)

                for k in range(K):
                    base = k * DN * L
                    xinT0 = cde.tile([128, L], bf16, tag="xinT0")
                    nc.sync.dma_start_transpose(xinT0[:], AP(xs_hbm, base, [[DN, L], [1, 128]]))
                    xinT1 = cde.tile([64, L], bf16, tag="xinT1")
                    nc.sync.dma_start_transpose(xinT1[:], AP(xs_hbm, base + 128, [[DN, L], [1, 64]]))
                    bc = cde.tile([32, L], bf16, tag="bc")
                    for fc in range(NLC):
                        fs = slice(fc * LC, (fc + 1) * LC)
                        ps = mp.tile([38, LC], f32, tag="mmA")
                        nc.tensor.matmul(ps[:], wl0[:, k * 38:(k + 1) * 38], xinT0[:, fs],
                                         start=True, stop=False)
                        nc.tensor.matmul(ps[:], wl1[:, k * 38:(k + 1) * 38], xinT1[:, fs],
                                         start=False, stop=True)
                        nc.scalar.activation(bc[:, fs], ps[0:32, :], AF.Identity,
                                             bias=pc(Cols.bc_bias[k], 32),
                                             scale=pc(Cols.bc_scale[k], 32))
                        d6c = cde2.tile([R, LC], f32, tag="d6c")
                        nc.scalar.activation(d6c[:], ps[32:38, :], AF.Identity,
                                             bias=pc(Cols.d6_bias[k], 6),
                                             scale=pc(Cols.d6_scale[k], 6))
                        d01c = cde2.tile([R, LC], bf16, tag="d01c")
                        nc.vector.tensor_scalar(d01c[:], d6c[:], 0.0, None, op0=OP.is_gt)
                        nc.sync.dma_start(dts_hbm[k, :, fs], d01c[:])
                    d01 = cde.tile([R, L], bf16, tag="d01")
                    nc.sync.dma_start(d01[:], AP(dts_hbm, k * R * L, [[1, R], [R, L]]))
                    delta = cde.tile([96, L], bf16, tag="delta")
                    for fc in range(NLC):
                        fs = slice(fc * LC, (fc + 1) * LC)
                        ps = mp.tile([96, LC], f32, tag="mmA")
                        nc.tensor.matmul(ps[:], wdsb[:, k * DH:(k + 1) * DH], d01[:, fs],
                                         start=True, stop=True)
                        ev = cde2.tile([96, LC], bf16, tag="ev")
                        nc.scalar.activation(ev[:], ps[:], AF.Exp,
                                             bias=pc(Cols.dts_bias[k], 96),
                                             scale=pc(Cols.dts_scale[k], 96))
                        nc.scalar.activation(delta[:, fs], ev[:], AF.Ln, bias=1.0)
                    du = cde.tile([96, L], bf16, tag="du")
                    nc.vector.tensor_mul(du[:], delta[:], uks[k][:])
                    brep = cde.tile([128, L], bf16, tag="brep")
                    crep = cde.tile([128, L], bf16, tag="crep")
                    for i in range(8):
                        _dma2(nc, brep[16 * i:16 * i + 16, :], bc[0:16, :])
                        _dma2(nc, crep[16 * i:16 * i + 16, :], bc[16:32, :])

                    yksb = cde.tile([96, L], f32, tag="yksb")
                    hcarry = {}
                    for lh in range(2):
                        for t in range(NT):
                            drep = cde2.tile([128, L // 2], bf16, tag="drep")
                            nc.sync.dma_start(
                                drep[:], delta[8 * t:8 * t + 8, lh * 2048:(lh + 1) * 2048]
                                .unsqueeze(1).broadcast_to([8, 16, 2048]))
                            durep = cde2.tile([128, L // 2], bf16, tag="durep")
                            nc.sync.dma_start(
                                durep[:], du[8 * t:8 * t + 8, lh * 2048:(lh + 1) * 2048]
                                .unsqueeze(1).broadcast_to([8, 16, 2048]))
                            hprev = None
                            for l4 in range(4):
                                lc = lh * 4 + l4
                                fs = slice(lc * LC, (lc + 1) * LC)
                                ls = slice(l4 * LC, (l4 + 1) * LC)
                                dA = sc3.tile([128, LC], bf16, tag="dA")
                                nc.scalar.activation(dA[:], drep[:, ls], AF.Exp,
                                                     scale=asb[:, k * NT + t:k * NT + t + 1])
                                inp = sc3.tile([128, LC], bf16, tag="inp")
                                nc.vector.tensor_mul(inp[:], durep[:, ls], brep[:, fs])
                                hh = sc3.tile([128, LC], bf16, tag="hh")
                                if lh == 0 and l4 == 0:
                                    init = 0.0
                                elif l4 == 0:
                                    init = hcarry[t][:, 0:1]
                                else:
                                    init = hprev[:, LC - 1:LC]
                                nc.vector.tensor_tensor_scan(hh[:], dA[:], inp[:], init,
                                                             op0=OP.mult, op1=OP.add)
                                if l4 == 3 and lh == 0:
                                    hc = cde2.tile([128, 1], bf16, tag=f"hc{t}")
                                    nc.vector.tensor_copy(hc[:], hh[:, LC - 1:LC])
                                    hcarry[t] = hc
                                yt = sc3.tile([128, LC], bf16, tag="yt")
                                nc.vector.tensor_mul(yt[:], hh[:], crep[:, fs])
                                pY = yp.tile([96, LC], f32, tag=f"yp{l4}")
                                nc.tensor.matmul(pY[:], big[:, 88 - 8 * t:184 - 8 * t], yt[:],
                                                 start=(t == 0), stop=(t == NT - 1))
                                if t == NT - 1:
                                    if k in (1, 3):
                                        ya = yksb[:]
                                        dstap = AP(ya.tensor, ya.offset + 8 * lc,
                                                   [ya.ap[0], [1, 8], [64, 64]])
                                        nc.scalar.activation(dstap, pY[:], AF.Copy)
                                    else:
                                        nc.scalar.activation(yksb[:, fs], pY[:], AF.Copy)
                                hprev = hh
                    dst = AP(yk_hbm, k * DH * L, [[L, DH], [1, L]])
                    sap = _revfree(yksb[:], L) if k >= 2 else yksb[:]
                    _dma2(nc, dst, sap)
                sc3cm.__exit__(None, None, None)

            # ---- phase F: combine + LN + gate + out-proj (chunked) --------
            with tc.tile_pool(name="fin", bufs=1) as fp, \
                 tc.tile_pool(name="fin2", bufs=2) as f2:
                ysum = fp.tile([96, L], f32)
                _dma2(nc, ysum[:], AP(yk_hbm, 0, [[L, DH], [1, L]]))
                for k in range(1, K):
                    yld = fp.tile([96, L], f32, tag="yld")
                    _dma2(nc, yld[:], AP(yk_hbm, k * DH * L, [[L, DH], [1, L]]))
                    nc.vector.tensor_add(ysum[:], ysum[:], yld[:])
                nc.vector.scalar_tensor_tensor(ysum[:], xconv[:], pc(Cols.dsum, 96), ysum[:],
                                               op0=OP.mult, op1=OP.add)
                _dma2(nc, cc2_in[:], ysum[:])
                nc.gpsimd.collective_compute("AllGather", OP.bypass, replica_groups=groups,
                                             ins=[cc2_in[:]], outs=[cc2_out[:]])
                osb = fp.tile([48, L], f32)
                for fc in range(NLC):
                    fs = slice(fc * LC, (fc + 1) * LC)
                    yc0 = f2.tile([128, LC], f32, tag="yc0")
                    yc1 = f2.tile([64, LC], f32, tag="yc1")
                    nc.sync.dma_start(yc0[:], cc2_out[0:128, fs])
                    nc.sync.dma_start(yc1[:], cc2_out[128:DN, fs])
                    yb0 = f2.tile([128, LC], bf16, tag="yb0")
                    yb1 = f2.tile([64, LC], bf16, tag="yb1")
                    nc.vector.tensor_copy(yb0[:], yc0[:])
                    nc.vector.tensor_copy(yb1[:], yc1[:])
                    sq0 = f2.tile([128, LC], bf16, tag="sq0")
                    sq1 = f2.tile([64, LC], bf16, tag="sq1")
                    nc.scalar.square(sq0[:], yb0[:])
                    nc.scalar.square(sq1[:], yb1[:])
                    mu = f2.tile([1, LC], f32, tag="mu")
                    sq = f2.tile([1, LC], f32, tag="sq")
                    for dst, a0, a1 in ((mu, yb0, yb1), (sq, sq0, sq1)):
                        ps = mp.tile([1, LC], f32, tag="mmA")
                        nc.tensor.matmul(ps[:], onesb[:], a0[:], start=True, stop=False)
                        nc.tensor.matmul(ps[:], onesb[0:64, :], a1[:],
                                         start=False, stop=True)
                        nc.scalar.activation(dst[:], ps[:], AF.Copy)
                    nc.vector.tensor_scalar(mu[:], mu[:], 1.0 / DN, None, op0=OP.mult)
                    mu2 = f2.tile([1, LC], f32, tag="mu2")
                    nc.vector.tensor_mul(mu2[:], mu[:], mu[:])
                    nc.vector.tensor_scalar(sq[:], sq[:], 1.0 / DN, None, op0=OP.mult)
                    nc.vector.tensor_sub(sq[:], sq[:], mu2[:])
                    nc.vector.tensor_scalar(sq[:], sq[:], EPS, None, op0=OP.add)
                    ssb = f2.tile([1, LC], bf16, tag="ssb")
                    nc.scalar.activation(ssb[:], sq[:], AF.Sqrt)
                    mub = f2.tile([1, LC], bf16, tag="mub")
                    nc.vector.tensor_copy(mub[:], mu[:])
                    bcmu = f2.tile([128, LC], f32, tag="bcmu")
                    bcs = f2.tile([128, LC], f32, tag="bcs")
                    for dst, srcb in ((bcmu, mub), (bcs, ssb)):
                        ps = mp.tile([128, LC], f32, tag="mmA")
                        nc.tensor.matmul(ps[:], onebc[:], srcb[:], start=True, stop=True)
                        nc.scalar.activation(dst[:], ps[:], AF.Copy)
                    g0 = f2.tile([128, LC], bf16, tag="g0")
                    g1 = f2.tile([64, LC], bf16, tag="g1")
                    for gi in range(2):
                        yfc, nrow = ((yc0, 128), (yc1, 64))[gi]
                        t1 = f2.tile([nrow, LC], f32, tag=f"lnt{gi}")
                        nc.vector.tensor_sub(t1[:], yfc[:], bcmu[0:nrow, :])
                        nc.vector.tensor_scalar(t1[:], t1[:], pc(Cols.lnw[gi], nrow),
                                                None, op0=OP.mult)
                        nc.vector.scalar_tensor_tensor(t1[:], bcs[0:nrow, :],
                                                       pc(Cols.lnb[gi], nrow), t1[:],
                                                       op0=OP.mult, op1=OP.add)
                        tg = f2.tile([nrow, LC], bf16, tag=f"tg{gi}")
                        nc.vector.tensor_scalar(tg[:], t1[:], 0.0, None, op0=OP.is_gt)
                        zt = (z0, z1)[gi]
                        g = (g0, g1)[gi]
                        nc.vector.tensor_tensor(g[:], tg[:], zt[:, fs], op=OP.is_equal)
                    ps = mp.tile([48, LC], f32, tag="mmA")
                    nc.tensor.matmul(ps[:], wo0[:], g0[:], start=True, stop=False)
                    nc.tensor.matmul(ps[:], wo1[:], g1[:], start=False, stop=True)
                    nc.scalar.activation(osb[:, fs], ps[:], AF.Identity,
                                         bias=pc(Cols.out_bias, 48),
                                         scale=pc(Cols.out_scale, 48))
                _dma2(nc, out_h[:], osb[:])

    nc.compile()
    return nc


# ------------------------------------------------------------------ host prep
def _sgn(x):
    return np.where(x >= 0, 1.0, -1.0).astype(np.float32)


def _prep_core(inputs, c):
    b, hf = c // 2, c % 2
    mine = np.arange(hf * DH, hf * DH + DH)
    perm = np.concatenate([mine, np.arange((1 - hf) * DH, (1 - hf) * DH + DH)])
    bf = ml_dtypes.bfloat16
    d = {}
    d["x_t"] = np.ascontiguousarray(
        inputs["x"][b].reshape(L, 96).T).astype(np.float32)

    W_in, s_in = inputs["W_in"], inputs["s_in"]
    sig_in = _sgn(W_in - W_in.mean(1, keepdims=True))
    row_order = np.concatenate([perm, 192 + np.arange(192)])
    sig_o = sig_in[row_order]
    s_o = s_in[row_order, 0]
    d["w_in_T"] = np.ascontiguousarray(sig_o.T).astype(bf)
    in_scale_full = 2.0 * s_o
    in_bias_full = inputs["b_in"][row_order] - s_o * sig_o.sum(1)

    conv_W = inputs["conv_W"]
    sc = np.abs(conv_W).mean(axis=(1, 2, 3))[mine]
    sig_c = _sgn(conv_W[mine])
    clhsT = np.zeros((9, DN, DH), np.float32)
    for ky in range(3):
        for kx in range(3):
            clhsT[ky * 3 + kx] = sig_c[:, perm, ky, kx].T
    d["conv_lhsT"] = clhsT.astype(bf)
    conv_scale = 2.0 * sc
    conv_bias = (inputs["conv_b"][mine] + inputs["rp_b0"][mine]
                 - sc * sig_c.reshape(DH, -1).sum(1))

    Wl, sl, bl = inputs["Wl"], inputs["sl"], inputs["bl"]
    reord = np.concatenate([np.arange(R, R + 32), np.arange(R)])
    wlT = np.zeros((K, DN, 38), np.float32)
    bc_scale = np.zeros((K, 32), np.float32)
    bc_bias = np.zeros((K, 32), np.float32)
    d6_scale = np.zeros((K, R), np.float32)
    d6_bias = np.zeros((K, R), np.float32)
    for k in range(K):
        sig = _sgn(Wl[k] - Wl[k].mean(1, keepdims=True))[reord]
        s_ = sl[k][reord, 0]
        b_ = bl[k][reord]
        wlT[k] = sig.T
        sc_ = 2.0 * s_
        bi_ = b_ - s_ * sig.sum(1)
        bc_scale[k], bc_bias[k] = sc_[0:32], bi_[0:32]
        d6_scale[k], d6_bias[k] = sc_[32:38], bi_[32:38]
    d["wl_T"] = wlT.astype(bf)

    Wd, sd, bd = inputs["Wd"], inputs["sd"], inputs["bd"]
    dt_bias = inputs["dt_bias"].reshape(K, DN)[:, mine]
    wdT = np.zeros((K, R, DH), np.float32)
    dts_scale = np.zeros((K, DH), np.float32)
    dts_bias = np.zeros((K, DH), np.float32)
    for k in range(K):
        sig = _sgn(Wd[k] - Wd[k].mean(1, keepdims=True))[mine]
        s_ = sd[k][mine, 0]
        wdT[k] = sig.T
        dts_scale[k] = 2.0 * s_
        dts_bias[k] = bd[k][mine] - s_ * sig.sum(1) + dt_bias[k]
    d["wd_T"] = wdT.astype(bf)

    A = -np.exp(inputs["A_logs"]).reshape(K, DN, N)[:, mine]
    ap = np.zeros((128, K * NT), np.float32)
    for k in range(K):
        for t in range(NT):
            ap[:, k * NT + t] = A[k, 8 * t:8 * t + 8, :].reshape(128)
    d["a_pat"] = ap

    W_out, s_out, b_out = inputs["W_out"], inputs["s_out"], inputs["b_out"]
    sig_out = _sgn(W_out - W_out.mean(1, keepdims=True))
    mo = np.arange(hf * 48, hf * 48 + 48)
    d["wout_T"] = np.ascontiguousarray(sig_out[mo].T).astype(bf)
    out_scale = 2.0 * s_out[mo, 0]
    out_bias = b_out[mo] - s_out[mo, 0] * sig_out[mo].sum(1)

    bp = np.zeros((128, 184), np.float32)
    for j in range(8):
        bp[j * 16:(j + 1) * 16, 88 + j] = 1.0
    d["bigpat"] = bp.astype(bf)

    P = np.zeros((128, Cols.NCOL), np.float32)
    for i in range(4):
        n_ = (128, 64, 128, 64)[i]
        o_ = (0, 128, 192, 320)[i]
        P[0:n_, Cols.in_scale[i]] = in_scale_full[o_:o_ + n_]
        P[0:n_, Cols.in_bias[i]] = in_bias_full[o_:o_ + n_]
    nm0 = -inputs["move0_b"][perm]
    P[0:128, Cols.neg_move0[0]] = nm0[0:128]
    P[0:64, Cols.neg_move0[1]] = nm0[128:192]
    P[0:96, Cols.conv_scale] = conv_scale
    P[0:96, Cols.conv_bias] = conv_bias
    P[0:96, Cols.prelu] = inputs["prelu_a"][mine]
    P[0:96, Cols.rp_b1] = inputs["rp_b1"][mine]
    for k in range(K):
        P[0:96, Cols.dts_scale[k]] = dts_scale[k]
        P[0:96, Cols.dts_bias[k]] = dts_bias[k]
        P[0:32, Cols.bc_scale[k]] = bc_scale[k]
        P[0:32, Cols.bc_bias[k]] = bc_bias[k]
        P[0:6, Cols.d6_scale[k]] = d6_scale[k]
        P[0:6, Cols.d6_bias[k]] = d6_bias[k]
    P[0:96, Cols.dsum] = inputs["Ds"].reshape(K, DN)[:, mine].sum(0)
    lnw, lnb = inputs["ln_w"], inputs["ln_b"]
    P[0:128, Cols.lnw[0]] = lnw[0:128]
    P[0:64, Cols.lnw[1]] = lnw[128:192]
    P[0:128, Cols.lnb[0]] = lnb[0:128]
    P[0:64, Cols.lnb[1]] = lnb[128:192]
    P[0:48, Cols.out_scale] = out_scale
    P[0:48, Cols.out_bias] = out_bias
    d["params"] = P
    return d


def kernel(**inputs):
    if "nc" not in _cache:
        _cache["nc"] = _build_nc()
    nc = _cache["nc"]
    inputs = {k: np.asarray(v) for k, v in inputs.items()}
    in_maps = [_prep_core(inputs, c) for c in range(8)]
    import os, time
    reps = int(os.environ.get("KREPS", "1"))
    res = run_bass_kernel_spmd(nc, in_maps, core_ids=list(range(8)))
    if reps > 1:
        ts = []
        for _ in range(reps):
            t0 = time.perf_counter()
            res = run_bass_kernel_spmd(nc, in_maps, core_ids=list(range(8)))
            ts.append(time.perf_counter() - t0)
        print(f"spmd wall times (ms): {[round(1e3 * t, 2) for t in ts]}")
        print(f"HW exec time: {int(min(ts) * 1e9)} ns (upper bound, incl dispatch)")
    _cache["res"] = res
    out = np.zeros((B, L, 96), np.float32)
    for c in range(8):
        b, hf = c // 2, c % 2
        oh = res.results[c]["out_h"]
        out[b, :, hf * 48:(hf + 1) * 48] = oh.T
    return out.reshape(B, H, W, 96)
